# revision 1
# baseline (speedup 1.0000x reference)
"""Multi-head causal self-attention (B=2, L=2048, D=1024, H=16) on 8 TRN2
NeuronCores.

Sharding: core c handles batch b = c // 4 and head group g = c % 4 (4 heads,
i.e. a 256-wide slice of the QKV output dim and the matching 256 rows of
Wo^T).  Each core computes a full (L, D) partial of the output projection;
the host sums the 4 partials per batch and adds bo.

On-core layout (f32 everywhere, f32r = tf32 for matmul inputs):
  XT  [128, 8, 2048]   x^T  (d-chunk on partitions), scoped pool, via PE
  WqT/WkT/WvT [128, 8, 256]  W^T, scoped pool, via PE transpose
  QT/KT [128, 2, 2048] q^T / k^T (dq on partitions, chunk = head pair)
  KTz [128, 4, 2048]   k^T zero-padded per head to K=128 rows: the PE HAM
                       clock gate only un-throttles (1.2 -> 2.4 GHz) when
                       matmuls stream all 128 partitions, and K=64 f32r
                       matmuls are also ~1.6x slower per column (measured).
  Vp  [128, 16, 4, 65] v natural + ones column (softmax denominator trick)
  OT  [128, 2, 2048]   attention out^T, normalized in place
  WoT [128, 2, 1024]   Wo^T slice (built after the projection pool closes)

Attention per (qt, head): s^T[k, q] = KTz_h . QT_pair with zero rows killing
the other head; exp on ACT from a 2-bank PSUM pair; causal mask via gpsimd
affine_select on diagonal tiles; o^T + denominator accumulated in PSUM with
V'; normalize = PE ones-broadcast of the denominator + DVE reciprocal of the
broadcast (64 parallel lanes) + one multiply, emitted one tile late so the
PE stream never waits. Output projection is woven in per 512-row q block.
"""

import sys

for _p in ("/opt/trn_rl_repo", "/root/.axon_site/_ro/trn_rl_repo"):
    if _p not in sys.path:
        sys.path.append(_p)

from contextlib import ExitStack

import numpy as np

import concourse.bass as bass
import concourse.tile as tile
from concourse import bacc, mybir
from concourse.bass_utils import run_bass_kernel_spmd
from concourse.masks import make_identity

F32 = mybir.dt.float32
F32R = mybir.dt.float32r
F16 = mybir.dt.float16

B, L, D, H = 2, 2048, 1024, 16
DK = D // H  # 64
NCORES = 8
GH = 4  # heads per core
C = GH * DK  # 256: per-core slice of the qkv/head dim
QT_TILES = L // 512  # 4
KT_TILES = L // 128  # 16
DCH = D // 128  # 8


def _transpose_block(nc, psum, dst, w_sb, ident, n_chunks, col0):
    """PE-transpose `n_chunks` 128x128 blocks of w_sb into dst[:, chunk, col0:]."""
    for half in range(0, n_chunks, 4):
        n = min(4, n_chunks - half)
        pt = psum.tile([128, 512], F32, tag="ps", name="pt")
        for i in range(n):
            dci = half + i
            nc.tensor.matmul(
                pt[:, i * 128 : (i + 1) * 128],
                lhsT=w_sb[:, dci * 128 : (dci + 1) * 128],
                rhs=ident[:],
                is_transpose=True,
                start=(i == 0),
                stop=(i == n - 1),
            )
        nc.vector.tensor_copy(
            dst[:, half : half + n, col0 : col0 + 128],
            pt[:, : n * 128].rearrange("p (c q) -> p c q", c=n),
        )


def _build_program():
    nc = bacc.Bacc("TRN2", target_bir_lowering=False, debug=False, num_devices=NCORES)

    x_d = nc.dram_tensor("x", [L, D], F32, kind="ExternalInput").ap()
    wq_d = nc.dram_tensor("wq", [C, D], F32, kind="ExternalInput").ap()
    wk_d = nc.dram_tensor("wk", [C, D], F32, kind="ExternalInput").ap()
    wv_d = nc.dram_tensor("wv", [C, D], F32, kind="ExternalInput").ap()
    wo_d = nc.dram_tensor("wo", [D, C], F32, kind="ExternalInput").ap()
    bq_d = nc.dram_tensor("bq", [C], F32, kind="ExternalInput").ap()
    bk_d = nc.dram_tensor("bk", [C], F32, kind="ExternalInput").ap()
    bv_d = nc.dram_tensor("bv", [C], F32, kind="ExternalInput").ap()
    out_d = nc.dram_tensor("out", [L, D], F32, kind="ExternalOutput").ap()

    with tile.TileContext(nc) as tc, ExitStack() as ctx:
        pool = ctx.enter_context(tc.tile_pool(name="persist", bufs=1))
        psum = ctx.enter_context(tc.tile_pool(name="psum", bufs=4, space="PSUM"))
        psum2 = ctx.enter_context(tc.tile_pool(name="psum2", bufs=2, space="PSUM"))
        cp = ctx.enter_context(tc.tile_pool(name="copies", bufs=3))

        ident = pool.tile([128, 128], F32)
        make_identity(nc, ident)

        ones_f32 = pool.tile([128, 128], F32)
        nc.gpsimd.memset(ones_f32[:], 1.0)
        ones_r = pool.tile([1, 128], F32R)
        nc.vector.tensor_copy(ones_r[:], ones_f32[0:1, :])

        bq_sb = pool.tile([128, 2], F32)
        nc.sync.dma_start(bq_sb[:], bq_d.rearrange("(c p) -> p c", p=128))
        bk_sb = pool.tile([128, 2], F32)
        nc.sync.dma_start(bk_sb[:], bk_d.rearrange("(c p) -> p c", p=128))
        bv_sb = pool.tile([1, 256], F32)
        nc.sync.dma_start(bv_sb[:], bv_d[None, :])
        bv_r = pool.tile([1, 256], F32R)
        nc.vector.tensor_copy(bv_r[:], bv_sb[:])

        QTs = [pool.tile([128, 2, 512], F16, name=f"QT{g}") for g in range(4)]
        KTzs = [pool.tile([128, GH, 512], F16, name=f"KTz{g}") for g in range(4)]
        zeros_f32 = pool.tile([64, 512], F32)
        nc.gpsimd.memset(zeros_f32[:], 0.0)
        for g in range(4):
            for h in range(GH):
                zp = 64 - 64 * (h % 2)
                nc.vector.tensor_copy(KTzs[g][zp : zp + 64, h, :], zeros_f32[:])
        Vps = [pool.tile([128, 4, GH, DK + 1], F16, name=f"Vp{g}") for g in range(4)]
        OTs = [pool.tile([128, 2, 512], F32R, name=f"OT{g}") for g in range(4)]

        with nc.allow_low_precision(reason="tf32 rounding for f32r matmul inputs"):
            # ======== phase A (scoped pool): x^T, W^T, q/k/v projections ====
            with tc.tile_pool(name="projs", bufs=1) as pp, tc.tile_pool(
                name="loads", bufs=4
            ) as lp:
                XT = pp.tile([128, DCH, L], F32R)
                for qt in range(KT_TILES):  # 16 q-tiles of 128
                    x_sb = lp.tile([128, D], F32, tag="ld")
                    for hh in range(2):
                        nc.sync.dma_start(
                            x_sb[:, hh * 512 : (hh + 1) * 512],
                            x_d[qt * 128 : (qt + 1) * 128, hh * 512 : (hh + 1) * 512],
                        )
                    _transpose_block(nc, psum, XT, x_sb, ident, DCH, qt * 128)

                WT = {}
                for name, w_d in (("q", wq_d), ("k", wk_d), ("v", wv_d)):
                    wt = pp.tile([128, DCH, C], F32R, name=f"W{name}T")
                    WT[name] = wt
                    for j in range(2):  # dq chunks of 128
                        w_sb = lp.tile([128, D], F32, tag="ld")
                        nc.sync.dma_start(w_sb[:], w_d[j * 128 : (j + 1) * 128, :])
                        _transpose_block(nc, psum, wt, w_sb, ident, DCH, j * 128)

                WoT = pool.tile([128, 2, D], F32R)
                for ei in range(DCH):  # 8 chunks of e (output dim)
                    w_sb = lp.tile([128, C], F32, tag="ldo", bufs=2)
                    nc.sync.dma_start(w_sb[:], wo_d[ei * 128 : (ei + 1) * 128, :])
                    _transpose_block(nc, psum, WoT, w_sb, ident, 2, ei * 128)

                # ---- Q^T, K^T projections ----
                for name, wt, b_sb in (("q", WT["q"], bq_sb), ("k", WT["k"], bk_sb)):
                    for j in range(2):
                        for qt in range(QT_TILES):
                            ps = psum.tile([128, 512], F32, tag="ps")
                            for dci in range(DCH):
                                nc.tensor.matmul(
                                    ps[:],
                                    lhsT=wt[:, dci, j * 128 : (j + 1) * 128],
                                    rhs=XT[:, dci, qt * 512 : (qt + 1) * 512],
                                    start=(dci == 0),
                                    stop=(dci == DCH - 1),
                                )
                            if name == "q":
                                nc.vector.tensor_tensor(
                                    QTs[qt][:, j, :],
                                    ps[:],
                                    b_sb[:, j, None].to_broadcast((128, 512)),
                                    mybir.AluOpType.add,
                                )
                            else:
                                # write k^T split into the zero-padded KTz
                                for half in range(2):
                                    hp = 64 * half
                                    nc.vector.tensor_tensor(
                                        KTzs[qt][hp : hp + 64, 2 * j + half, :],
                                        ps[hp : hp + 64, :],
                                        b_sb[hp : hp + 64, j, None].to_broadcast(
                                            (64, 512)
                                        ),
                                        mybir.AluOpType.add,
                                    )

                # ---- V projection (natural layout + ones col) ----
                for kt in range(KT_TILES):
                    ps = psum.tile([128, 512], F32, tag="ps")
                    for dci in range(DCH):
                        nc.tensor.matmul(
                            ps[:, :256],
                            lhsT=XT[:, dci, kt * 128 : (kt + 1) * 128],
                            rhs=WT["v"][:, dci, :],
                            start=(dci == 0),
                            stop=False,
                        )
                    nc.tensor.matmul(
                        ps[:, :256], lhsT=ones_r[:], rhs=bv_r[:], start=False, stop=True
                    )
                    nc.vector.tensor_copy(
                        Vps[kt // 4][:, kt % 4, :, 0:DK],
                        ps[:, :256].rearrange("p (h d) -> p h d", h=GH),
                    )
                for g in range(4):
                    nc.vector.tensor_copy(
                        Vps[g][:, :, :, DK : DK + 1], ones_f32[:, 0:16]
                    )

            yp = ctx.enter_context(tc.tile_pool(name="youts", bufs=2))


            # ======== phase C: attention + woven output projection ========
            def normalize(h, qt, pso):
                hj, hp = h // 2, 64 * (h % 2)
                den_r = cp.tile([1, 512], F32R, tag="den", bufs=2)
                nc.vector.tensor_copy(den_r[:], pso[64:65, :])
                psb = psum.tile([128, 512], F32, tag="ps")
                nc.tensor.matmul(
                    psb[:64], lhsT=ones_r[:, 0:64], rhs=den_r[:], start=True, stop=True
                )
                rb = cp.tile([64, 512], F32, tag="rb", bufs=2)
                nc.vector.reciprocal(rb[:], psb[:64])
                nc.vector.tensor_tensor(
                    OTs[qt][hp : hp + 64, hj, :],
                    pso[:64],
                    rb[:],
                    mybir.AluOpType.mult,
                )

            def outproj(qt512):
                # project q rows [qt512*512, +512) and DMA them out; woven
                # into the next q-tile's attention so PE stays dense and the
                # output DMA is spread across the kernel.
                for sub in range(4):
                    q0 = qt512 * 512 + sub * 128
                    for e in range(2):
                        psy = psum.tile([128, 512], F32, tag="ps")
                        for cj in range(2):
                            nc.tensor.matmul(
                                psy[:],
                                lhsT=OTs[qt512][:, cj, sub * 128 : (sub + 1) * 128],
                                rhs=WoT[:, cj, e * 512 : (e + 1) * 512],
                                start=(cj == 0),
                                stop=(cj == 1),
                            )
                        y_sb = yp.tile([128, 512], F32, tag="y")
                        nc.vector.tensor_copy(y_sb[:], psy[:])
                        nc.sync.dma_start(
                            out_d[q0 : q0 + 128, e * 512 : (e + 1) * 512], y_sb[:]
                        )

            pending = None
            for qt in range(QT_TILES):
                n_kt = 4 * qt + 4
                for h in range(GH):
                    hj, hp = h // 2, 64 * (h % 2)
                    pso = psum.tile([128, 512], F32, tag="ps")
                    for kt2 in range(n_kt // 2):  # k-tile pairs share a
                        pss = psum2.tile([128, 1024], F32, tag="ps2")  # 2-bank psum
                        for i in range(2):
                            kt = 2 * kt2 + i
                            nc.tensor.matmul(
                                pss[:, i * 512 : (i + 1) * 512],
                                lhsT=KTzs[kt // 4][
                                    :, h, (kt % 4) * 128 : (kt % 4 + 1) * 128
                                ],
                                rhs=QTs[qt][:, hj, :],
                                start=True,
                                stop=True,
                            )
                        p_sb = cp.tile([128, 1024], F16, tag="p", bufs=5)
                        nc.scalar.activation(
                            p_sb[:],
                            pss[:],
                            mybir.ActivationFunctionType.Exp,
                            scale=0.125,
                        )
                        for i in range(2):
                            kt = 2 * kt2 + i
                            if kt >= 4 * qt:  # diagonal overlap: causal mask
                                nc.gpsimd.affine_select(
                                    out=p_sb[:, i * 512 : (i + 1) * 512],
                                    in_=p_sb[:, i * 512 : (i + 1) * 512],
                                    pattern=[[1, 512]],
                                    compare_op=mybir.AluOpType.is_ge,
                                    fill=0.0,
                                    base=qt * 512 - kt * 128,
                                    channel_multiplier=-1,
                                )
                            nc.tensor.matmul(
                                pso[:65],
                                lhsT=Vps[kt // 4][:, kt % 4, h, :],
                                rhs=p_sb[:, i * 512 : (i + 1) * 512],
                                start=(kt == 0),
                                stop=(kt == n_kt - 1),
                            )
                        if kt2 == 0 and pending is not None:
                            normalize(*pending)  # previous tile, PE has work
                            pending = None
                    pending = (h, qt, pso)
                    if h == 0 and qt > 0:
                        outproj(qt - 1)  # previous q block fully normalized
            normalize(*pending)
            outproj(QT_TILES - 1)

    nc.compile()
    return nc


_NC_CACHE = None


def _get_program():
    global _NC_CACHE
    if _NC_CACHE is None:
        _NC_CACHE = _build_program()
    return _NC_CACHE


def _run(in_maps, trace=False, **kw):
    nc = _get_program()
    return run_bass_kernel_spmd(nc, in_maps, list(range(NCORES)), trace=trace, **kw)


def _make_in_maps(x, Wq, bq, Wk, bk, Wv, bv, Wo, bo):
    a = lambda v: np.ascontiguousarray(np.asarray(v, dtype=np.float32))
    in_maps = []
    for core in range(NCORES):
        b, g = divmod(core, 4)
        s = slice(g * C, (g + 1) * C)
        in_maps.append(
            {
                "x": a(x[b]),
                "wq": a(Wq[s, :]),
                "wk": a(Wk[s, :]),
                "wv": a(Wv[s, :]),
                "wo": a(Wo[:, s]),
                "bq": a(bq[s]),
                "bk": a(bk[s]),
                "bv": a(bv[s]),
            }
        )
    return in_maps


def kernel(x, Wq, bq, Wk, bk, Wv, bv, Wo, bo, _trace=False, _trace_out=None, _tmpdir=None):
    in_maps = _make_in_maps(x, Wq, bq, Wk, bk, Wv, bv, Wo, bo)
    res = _run(in_maps, trace=_trace, tmpdir=_tmpdir)
    if _trace_out is not None:
        _trace_out.append(res)
    bo = np.asarray(bo, dtype=np.float32)
    out = np.empty((B, L, D), dtype=np.float32)
    for b in range(B):
        acc = res.results[4 * b]["out"].astype(np.float32)
        for g in range(1, 4):
            acc = acc + res.results[4 * b + g]["out"]
        out[b] = acc + bo[None, :]
    return out



# revision 3
# speedup vs baseline: 1.3760x; 1.3760x over previous
"""Multi-head causal self-attention (B=2, L=2048, D=1024, H=16) on 8 TRN2
NeuronCores.

Sharding: core c handles batch b = c // 4 and head group g = c % 4 (4 heads,
i.e. a 256-wide slice of the QKV output dim and the matching 256 rows of
Wo^T).  Each core computes a full (L, D) partial of the output projection;
the host sums the 4 partials per batch and adds bo.

v2 design (vs the PE-transpose baseline):
  - x and all weights are pre-transposed AND cast to f16 on the host, so
    x^T / W^T DMA straight into SBUF: no PE transposes, no DVE casts, half
    the input DMA bytes.  f16 rounding of the inputs costs ~1e-3 rel err
    (tolerance is 2e-2).
  - attention inner loop is software-pipelined one k-pair ahead: scores for
    pair n+1 are emitted before the PV matmuls of pair n, so the PE never
    sits out the exp (ACT) latency.  PE runs dense -> HAM clock stays at
    2.4 GHz.
  - softmax denominators via ones-column PV trick (row 64 of pso); the
    reciprocal uses DVE reciprocal_approx_fast (~5x cheaper than the exact
    reciprocal; denominators are benign positives).
  - output partials are written as f16 (host upcasts and sums).

On-core layout (f16 compute, f32 PSUM accumulation):
  XT  [128, 8, 2048]  x^T, DMA'd directly (scoped pool, freed after projs)
  WqT/WkT/WvT [128, 8, 256], WoT [128, 2, 1024], DMA'd directly
  QT  [128, 2, 512] x4 (dq on partitions, chunk = head pair)
  KTz [128, 4, 512] x4  k^T zero-padded per head to 128 rows: the PE HAM
                      clock gate only un-throttles at 128-partition matmuls
  Vp  [128, 4, 4, 65] x4  v natural + ones column (denominator trick)
  OT  [128, 2, 512] x4  normalized attention out^T
"""

import sys

for _p in ("/opt/trn_rl_repo", "/root/.axon_site/_ro/trn_rl_repo"):
    if _p not in sys.path:
        sys.path.append(_p)

from contextlib import ExitStack

import numpy as np

import concourse.bass as bass
import concourse.tile as tile
from concourse import bacc, mybir
from concourse.bass_utils import run_bass_kernel_spmd

F32 = mybir.dt.float32
F32R = mybir.dt.float32r
F16 = mybir.dt.float16

B, L, D, H = 2, 2048, 1024, 16
DK = D // H  # 64
NCORES = 8
GH = 4  # heads per core
C = GH * DK  # 256: per-core slice of the qkv/head dim
QT_TILES = L // 512  # 4
KT_TILES = L // 128  # 16
DCH = D // 128  # 8


def _build_program():
    nc = bacc.Bacc("TRN2", target_bir_lowering=False, debug=False, num_devices=NCORES)

    xt_d = nc.dram_tensor("xt", [D, L], F16, kind="ExternalInput").ap()
    wqt_d = nc.dram_tensor("wqt", [D, C], F16, kind="ExternalInput").ap()
    wkt_d = nc.dram_tensor("wkt", [D, C], F16, kind="ExternalInput").ap()
    wvt_d = nc.dram_tensor("wvt", [D, C], F16, kind="ExternalInput").ap()
    wot_d = nc.dram_tensor("wot", [C, D], F16, kind="ExternalInput").ap()
    bq_d = nc.dram_tensor("bq", [C], F32, kind="ExternalInput").ap()
    bk_d = nc.dram_tensor("bk", [C], F32, kind="ExternalInput").ap()
    bv_d = nc.dram_tensor("bv", [C], F16, kind="ExternalInput").ap()
    out_d = nc.dram_tensor("out", [L, D], F16, kind="ExternalOutput").ap()

    with tile.TileContext(nc) as tc, ExitStack() as ctx:
        pool = ctx.enter_context(tc.tile_pool(name="persist", bufs=1))
        psum = ctx.enter_context(tc.tile_pool(name="psum", bufs=4, space="PSUM"))
        psum2 = ctx.enter_context(tc.tile_pool(name="psum2", bufs=2, space="PSUM"))
        cp = ctx.enter_context(tc.tile_pool(name="copies", bufs=3))

        ones_f = pool.tile([1, 64], F32)
        nc.gpsimd.memset(ones_f[:], 1.0)
        ones_r = pool.tile([1, 64], F32R)
        nc.vector.tensor_copy(ones_r[:], ones_f[:])
        ones_h = pool.tile([1, 128], F16)
        nc.gpsimd.memset(ones_h[:], 1.0)

        bq_sb = pool.tile([128, 2], F32)
        nc.sync.dma_start(bq_sb[:], bq_d.rearrange("(c p) -> p c", p=128))
        bk_sb = pool.tile([128, 2], F32)
        nc.sync.dma_start(bk_sb[:], bk_d.rearrange("(c p) -> p c", p=128))
        bv_sb = pool.tile([1, 256], F16)
        nc.sync.dma_start(bv_sb[:], bv_d[None, :])

        QTs = [pool.tile([128, 2, 512], F16, name=f"QT{g}") for g in range(4)]
        KTzs = [pool.tile([128, GH, 512], F16, name=f"KTz{g}") for g in range(4)]
        for g in range(4):
            nc.gpsimd.memset(KTzs[g][:], 0.0)
        Vps = [pool.tile([128, 4, GH, DK + 1], F16, name=f"Vp{g}") for g in range(4)]
        for g in range(4):
            nc.gpsimd.memset(Vps[g][:, :, :, DK : DK + 1], 1.0)
        OTs = [pool.tile([128, 2, 512], F16, name=f"OT{g}") for g in range(4)]
        WoT = pool.tile([128, 2, D], F16)

        with nc.allow_low_precision(reason="f16 activations/weights; f32 psum"):
            # ======== phase A (scoped pool): load x^T / W^T, q/k/v projs ====
            with tc.tile_pool(name="projs", bufs=1) as pp:
                XT = pp.tile([128, DCH, L], F16)
                WT = {}
                for name, w_d in (("q", wqt_d), ("k", wkt_d), ("v", wvt_d)):
                    WT[name] = pp.tile([128, DCH, C], F16, name=f"W{name}T")
                # x chunk 0 + Wq first so the first psum group starts early
                nc.sync.dma_start(XT[:, 0, :], xt_d[0:128, :])
                for c in range(DCH):
                    nc.sync.dma_start(WT["q"][:, c, :], wqt_d[c * 128 : (c + 1) * 128, :])
                for c in range(1, DCH):
                    nc.sync.dma_start(XT[:, c, :], xt_d[c * 128 : (c + 1) * 128, :])
                for name, w_d in (("k", wkt_d), ("v", wvt_d)):
                    for c in range(DCH):
                        nc.sync.dma_start(
                            WT[name][:, c, :], w_d[c * 128 : (c + 1) * 128, :]
                        )
                for c in range(2):
                    nc.sync.dma_start(WoT[:, c, :], wot_d[c * 128 : (c + 1) * 128, :])

                # ---- Q^T, K^T projections ----
                for name, b_sb in (("q", bq_sb), ("k", bk_sb)):
                    wt = WT[name]
                    for j in range(2):
                        for qt in range(QT_TILES):
                            ps = psum.tile([128, 512], F32, tag="ps")
                            for dci in range(DCH):
                                nc.tensor.matmul(
                                    ps[:],
                                    lhsT=wt[:, dci, j * 128 : (j + 1) * 128],
                                    rhs=XT[:, dci, qt * 512 : (qt + 1) * 512],
                                    start=(dci == 0),
                                    stop=(dci == DCH - 1),
                                )
                            if name == "q":
                                nc.vector.tensor_tensor(
                                    QTs[qt][:, j, :],
                                    ps[:],
                                    b_sb[:, j, None].to_broadcast((128, 512)),
                                    mybir.AluOpType.add,
                                )
                            else:
                                # write k^T split into the zero-padded KTz
                                for half in range(2):
                                    hp = 64 * half
                                    nc.vector.tensor_tensor(
                                        KTzs[qt][hp : hp + 64, 2 * j + half, :],
                                        ps[hp : hp + 64, :],
                                        b_sb[hp : hp + 64, j, None].to_broadcast(
                                            (64, 512)
                                        ),
                                        mybir.AluOpType.add,
                                    )

                # ---- V projection (natural layout; ones col via memset) ----
                for kt in range(KT_TILES):
                    ps = psum.tile([128, 512], F32, tag="ps")
                    for dci in range(DCH):
                        nc.tensor.matmul(
                            ps[:, :256],
                            lhsT=XT[:, dci, kt * 128 : (kt + 1) * 128],
                            rhs=WT["v"][:, dci, :],
                            start=(dci == 0),
                            stop=False,
                        )
                    nc.tensor.matmul(
                        ps[:, :256], lhsT=ones_h[:], rhs=bv_sb[:], start=False, stop=True
                    )
                    nc.vector.tensor_copy(
                        Vps[kt // 4][:, kt % 4, :, 0:DK],
                        ps[:, :256].rearrange("p (h d) -> p h d", h=GH),
                    )

            yp = ctx.enter_context(tc.tile_pool(name="youts", bufs=2))

            # ======== phase B: attention, software-pipelined ========
            deferred = []  # weave queue: emitted one item per kt2 slot

            def normalize(h, qt, pso):
                hj, hp = h // 2, 64 * (h % 2)
                den_r = cp.tile([1, 512], F32R, tag="den", bufs=2)
                nc.vector.tensor_copy(den_r[:], pso[64:65, :])
                psb = psum.tile([128, 512], F32, tag="ps")
                nc.tensor.matmul(
                    psb[:64], lhsT=ones_r[:], rhs=den_r[:], start=True, stop=True
                )
                rb = cp.tile([64, 512], F32, tag="rb", bufs=2)
                nc.vector.reciprocal_approx_fast(rb[:], psb[:64])
                nc.vector.tensor_tensor(
                    OTs[qt][hp : hp + 64, hj, :],
                    pso[:64],
                    rb[:],
                    mybir.AluOpType.mult,
                )

            def outproj_unit(qt512, sub, e):
                q0 = qt512 * 512 + sub * 128
                psy = psum.tile([128, 512], F32, tag="ps")
                for cj in range(2):
                    nc.tensor.matmul(
                        psy[:],
                        lhsT=OTs[qt512][:, cj, sub * 128 : (sub + 1) * 128],
                        rhs=WoT[:, cj, e * 512 : (e + 1) * 512],
                        start=(cj == 0),
                        stop=(cj == 1),
                    )
                y_sb = yp.tile([128, 512], F16, tag="y")
                nc.vector.tensor_copy(y_sb[:], psy[:])
                nc.sync.dma_start(
                    out_d[q0 : q0 + 128, e * 512 : (e + 1) * 512], y_sb[:]
                )

            pend_pv = None  # (pso, p_sb, kt2, n_kt, h, qt)

            def emit_pv(state):
                pso, p_sb, kt2, n_kt, h, qt = state
                for i in range(2):
                    kt = 2 * kt2 + i
                    nc.tensor.matmul(
                        pso[:65],
                        lhsT=Vps[kt // 4][:, kt % 4, h, :],
                        rhs=p_sb[:, i * 512 : (i + 1) * 512],
                        start=(kt == 0),
                        stop=(kt == n_kt - 1),
                    )
                if kt2 == n_kt // 2 - 1:
                    deferred.append(("norm", (h, qt, pso)))
                    if h == GH - 1:
                        for sub in range(4):
                            for e in range(2):
                                deferred.append(("proj", (qt, sub, e)))

            def pop_deferred():
                if deferred:
                    kind, args = deferred.pop(0)
                    if kind == "norm":
                        normalize(*args)
                    else:
                        outproj_unit(*args)

            for qt in range(QT_TILES):
                n_kt = 4 * qt + 4
                for h in range(GH):
                    pso = psum.tile([128, 512], F32, tag="ps")
                    for kt2 in range(n_kt // 2):
                        pss = psum2.tile([128, 1024], F32, tag="ps2")
                        for i in range(2):
                            kt = 2 * kt2 + i
                            nc.tensor.matmul(
                                pss[:, i * 512 : (i + 1) * 512],
                                lhsT=KTzs[kt // 4][
                                    :, h, (kt % 4) * 128 : (kt % 4 + 1) * 128
                                ],
                                rhs=QTs[qt][:, h // 2, :],
                                start=True,
                                stop=True,
                            )
                        p_sb = cp.tile([128, 1024], F16, tag="p", bufs=5)
                        nc.scalar.activation(
                            p_sb[:],
                            pss[:],
                            mybir.ActivationFunctionType.Exp,
                            scale=0.125,
                        )
                        for i in range(2):
                            kt = 2 * kt2 + i
                            if kt >= 4 * qt:  # diagonal overlap: causal mask
                                nc.gpsimd.affine_select(
                                    out=p_sb[:, i * 512 : (i + 1) * 512],
                                    in_=p_sb[:, i * 512 : (i + 1) * 512],
                                    pattern=[[1, 512]],
                                    compare_op=mybir.AluOpType.is_ge,
                                    fill=0.0,
                                    base=qt * 512 - kt * 128,
                                    channel_multiplier=-1,
                                )
                        if pend_pv is not None:
                            emit_pv(pend_pv)
                            pop_deferred()
                        pend_pv = (pso, p_sb, kt2, n_kt, h, qt)
            emit_pv(pend_pv)
            while deferred:
                pop_deferred()

    nc.compile()
    return nc


_NC_CACHE = None


def _get_program():
    global _NC_CACHE
    if _NC_CACHE is None:
        _NC_CACHE = _build_program()
    return _NC_CACHE


def _run(in_maps, trace=False, **kw):
    nc = _get_program()
    return run_bass_kernel_spmd(nc, in_maps, list(range(NCORES)), trace=trace, **kw)


def _t16(a):
    return np.ascontiguousarray(np.asarray(a).T.astype(np.float16, order="C"))


def _make_in_maps(x, Wq, bq, Wk, bk, Wv, bv, Wo, bo):
    xts = [_t16(np.asarray(x)[b]) for b in range(B)]  # [D, L] f16
    in_maps = []
    for core in range(NCORES):
        b, g = divmod(core, 4)
        s = slice(g * C, (g + 1) * C)
        in_maps.append(
            {
                "xt": xts[b],
                "wqt": _t16(np.asarray(Wq)[s, :]),  # [D, C]
                "wkt": _t16(np.asarray(Wk)[s, :]),
                "wvt": _t16(np.asarray(Wv)[s, :]),
                "wot": _t16(np.asarray(Wo)[:, s]),  # [C, D]
                "bq": np.ascontiguousarray(np.asarray(bq)[s], dtype=np.float32),
                "bk": np.ascontiguousarray(np.asarray(bk)[s], dtype=np.float32),
                "bv": np.ascontiguousarray(np.asarray(bv)[s]).astype(np.float16),
            }
        )
    return in_maps


def kernel(x, Wq, bq, Wk, bk, Wv, bv, Wo, bo, _trace=False, _trace_out=None, _tmpdir=None):
    in_maps = _make_in_maps(x, Wq, bq, Wk, bk, Wv, bv, Wo, bo)
    res = _run(in_maps, trace=_trace, tmpdir=_tmpdir)
    if _trace_out is not None:
        _trace_out.append(res)
    bo = np.asarray(bo, dtype=np.float32)
    out = np.empty((B, L, D), dtype=np.float32)
    for b in range(B):
        acc = res.results[4 * b]["out"].astype(np.float32)
        for g in range(1, 4):
            acc = acc + res.results[4 * b + g]["out"].astype(np.float32)
        out[b] = acc + bo[None, :]
    return out


# revision 10
# speedup vs baseline: 1.4021x; 1.0189x over previous
"""Multi-head causal self-attention (B=2, L=2048, D=1024, H=16) on 8 TRN2
NeuronCores.

Sharding: core c handles batch b = c // 4 and head group g = c % 4 (4 heads,
i.e. a 256-wide slice of the QKV output dim and the matching 256 rows of
Wo^T).  Each core computes a full (L, D) partial of the output projection;
the host sums the 4 partials per batch and adds bo.

v3 design:
  - x and all weights are pre-transposed AND cast to f16 on the host, so
    x^T / W^T DMA straight into SBUF: no PE transposes, no DVE casts, half
    the input DMA bytes (f16 rounding ~1e-3 rel err; tolerance 2e-2).
  - projections accumulate into [128,1024] PSUM pairs, dci-outer for Q so
    the PE starts as soon as the first x^T chunk + Wq land.
  - attention inner loop is software-pipelined one k-pair ahead of the PV
    matmuls so the PE never waits out the exp (ACT) latency.
  - causal diagonal tiles are column-truncated for qt>=1: scores/exp/PV
    only cover valid q columns (widths 512/384/256/128); the causal mask
    shrinks to one 128-col affine_select per diagonal tile.  Diagonal
    pairs run FIRST within a head so the final (full-width) off-diagonal
    PV carries the PSUM stop flag over the whole pso region.
  - softmax: ones-column of V accumulates the denominator into pso row 64;
    reciprocal_approx_fast (DVE) + gpsimd partition_broadcast + one DVE
    multiply normalize off the PE's critical path.
  - dedicated PSUM pools: scores pairs (2x[128,1024]), pso (2x[128,512]),
    outproj psy (2x[128,512]) so an outproj allocation never blocks on a
    still-accumulating pso bank.
  - output partials are written as f16 (host upcasts and sums).
"""

import sys

for _p in ("/opt/trn_rl_repo", "/root/.axon_site/_ro/trn_rl_repo"):
    if _p not in sys.path:
        sys.path.append(_p)

from contextlib import ExitStack

import numpy as np

import concourse.bass as bass
import concourse.tile as tile
from concourse import bacc, mybir
from concourse.bass_utils import run_bass_kernel_spmd

F32 = mybir.dt.float32
F32R = mybir.dt.float32r
F16 = mybir.dt.float16

B, L, D, H = 2, 2048, 1024, 16
DK = D // H  # 64
NCORES = 8
GH = 4  # heads per core
C = GH * DK  # 256: per-core slice of the qkv/head dim
QT_TILES = L // 512  # 4
KT_TILES = L // 128  # 16
DCH = D // 128  # 8


def _build_program():
    nc = bacc.Bacc("TRN2", target_bir_lowering=False, debug=False, num_devices=NCORES)

    xt_d = nc.dram_tensor("xt", [D, L], F16, kind="ExternalInput").ap()
    wqt_d = nc.dram_tensor("wqt", [D, C], F16, kind="ExternalInput").ap()
    wkt_d = nc.dram_tensor("wkt", [D, C], F16, kind="ExternalInput").ap()
    wvt_d = nc.dram_tensor("wvt", [D, C], F16, kind="ExternalInput").ap()
    wot_d = nc.dram_tensor("wot", [C, D], F16, kind="ExternalInput").ap()
    bq_d = nc.dram_tensor("bq", [C], F32, kind="ExternalInput").ap()
    bk_d = nc.dram_tensor("bk", [C], F32, kind="ExternalInput").ap()
    bv_d = nc.dram_tensor("bv", [4 * C], F16, kind="ExternalInput").ap()
    out_d = nc.dram_tensor("out", [L, D], F16, kind="ExternalOutput").ap()

    with tile.TileContext(nc) as tc, ExitStack() as ctx:
        pool = ctx.enter_context(tc.tile_pool(name="persist", bufs=1))

        ones_h = pool.tile([1, 128], F16)
        nc.gpsimd.memset(ones_h[:], 1.0)
        ones_f = pool.tile([1, 64], F32)
        nc.gpsimd.memset(ones_f[:], 1.0)
        ones_r = pool.tile([1, 64], F32R)
        nc.vector.tensor_copy(ones_r[:], ones_f[:])

        bq_sb = pool.tile([128, 2], F32)
        nc.sync.dma_start(bq_sb[:], bq_d.rearrange("(c p) -> p c", p=128))
        bk_sb = pool.tile([128, 2], F32)
        nc.sync.dma_start(bk_sb[:], bk_d.rearrange("(c p) -> p c", p=128))
        bv_sb = pool.tile([1, 4 * C], F16)

        QT = pool.tile([128, 2, L], F16)
        KTzs = [pool.tile([128, GH, 512], F16, name=f"KTz{g}") for g in range(4)]
        for g in range(4):
            nc.gpsimd.memset(KTzs[g][:], 0.0)
        Vps = [pool.tile([128, 4, GH, DK + 1], F16, name=f"Vp{g}") for g in range(4)]
        for g in range(4):
            nc.gpsimd.memset(Vps[g][:, :, :, DK : DK + 1], 1.0)
        OTs = [pool.tile([128, 2, 512], F16, name=f"OT{g}") for g in range(4)]
        WoT = pool.tile([128, 2, D], F16)

        with nc.allow_low_precision(reason="f16 activations/weights; f32 psum"):
            # ======== phase A (scoped pools): load x^T / W^T, projections ===
            with tc.tile_pool(name="projs", bufs=1) as pp, tc.tile_pool(
                name="psA", bufs=4, space="PSUM"
            ) as psA:
                XT = pp.tile([128, DCH, L], F16)
                WT = {}
                for name in ("q", "k", "v"):
                    WT[name] = pp.tile([128, DCH, C], F16, name=f"W{name}T")
                # x chunk 0 + Wq first so the first Q psum group starts early
                nc.sync.dma_start(XT[:, 0, :], xt_d[0:128, :])
                for c in range(DCH):
                    nc.sync.dma_start(WT["q"][:, c, :], wqt_d[c * 128 : (c + 1) * 128, :])
                for c in range(1, DCH):
                    nc.sync.dma_start(XT[:, c, :], xt_d[c * 128 : (c + 1) * 128, :])
                for name, w_d in (("k", wkt_d), ("v", wvt_d)):
                    for c in range(DCH):
                        nc.sync.dma_start(
                            WT[name][:, c, :], w_d[c * 128 : (c + 1) * 128, :]
                        )
                nc.sync.dma_start(bv_sb[:], bv_d[None, :])
                for c in range(2):
                    nc.sync.dma_start(WoT[:, c, :], wot_d[c * 128 : (c + 1) * 128, :])

                # ---- Q^T / K^T: dci-outer over 4 [128,1024] accumulators ----
                for name, b_sb in (("q", bq_sb), ("k", bk_sb)):
                    wt = WT[name]
                    acc = [
                        psA.tile([128, 1024], F32, tag="psA", name=f"ps{name}{i}")
                        for i in range(4)
                    ]
                    for dci in range(DCH):
                        for j in range(2):
                            for qp in range(2):
                                for hf in range(2):
                                    nc.tensor.matmul(
                                        acc[2 * j + qp][:, hf * 512 : (hf + 1) * 512],
                                        lhsT=wt[:, dci, j * 128 : (j + 1) * 128],
                                        rhs=XT[
                                            :,
                                            dci,
                                            qp * 1024
                                            + hf * 512 : qp * 1024
                                            + (hf + 1) * 512,
                                        ],
                                        start=(dci == 0),
                                        stop=(dci == DCH - 1),
                                    )
                    for j in range(2):
                        for qp in range(2):
                            ps = acc[2 * j + qp]
                            if name == "q":
                                nc.vector.tensor_tensor(
                                    QT[:, j, qp * 1024 : (qp + 1) * 1024],
                                    ps[:],
                                    b_sb[:, j, None].to_broadcast((128, 1024)),
                                    mybir.AluOpType.add,
                                )
                            else:
                                for qh in range(2):
                                    qt = qp * 2 + qh
                                    for half in range(2):
                                        hp = 64 * half
                                        nc.vector.tensor_tensor(
                                            KTzs[qt][hp : hp + 64, 2 * j + half, :],
                                            ps[hp : hp + 64, qh * 512 : (qh + 1) * 512],
                                            b_sb[hp : hp + 64, j, None].to_broadcast(
                                                (64, 512)
                                            ),
                                            mybir.AluOpType.add,
                                        )

                # ---- V (natural layout): 4 kt per [128,1024] accumulator ----
                for g in range(4):
                    psv = psA.tile([128, 1024], F32, tag="psA", name=f"psV{g}")
                    for ksub in range(4):
                        kt = 4 * g + ksub
                        for dci in range(DCH):
                            # start only once per psum bank: the whole 2KB
                            # zero-region goes pending-zero, so the odd kt's
                            # first write overwrites (not accumulates) anyway
                            nc.tensor.matmul(
                                psv[:, ksub * 256 : (ksub + 1) * 256],
                                lhsT=XT[:, dci, kt * 128 : (kt + 1) * 128],
                                rhs=WT["v"][:, dci, :],
                                start=(dci == 0 and ksub % 2 == 0),
                                stop=False,
                            )
                    for hf in range(2):
                        nc.tensor.matmul(
                            psv[:, hf * 512 : (hf + 1) * 512],
                            lhsT=ones_h[:],
                            rhs=bv_sb[:, hf * 512 : (hf + 1) * 512],
                            start=False,
                            stop=True,
                        )
                    nc.vector.tensor_copy(
                        Vps[g][:, :, :, 0:DK],
                        psv[:].rearrange("p (k h d) -> p k h d", k=4, h=GH),
                    )

            # ======== attention pools (PSUM freed by psA close) ========
            pssP = ctx.enter_context(tc.tile_pool(name="pss", bufs=2, space="PSUM"))
            psoP = ctx.enter_context(tc.tile_pool(name="pso", bufs=2, space="PSUM"))
            psyP = ctx.enter_context(tc.tile_pool(name="psy", bufs=2, space="PSUM"))
            cp = ctx.enter_context(tc.tile_pool(name="copies", bufs=3))
            yp = ctx.enter_context(tc.tile_pool(name="youts", bufs=2))

            deferred = []  # weave queue: emitted one item per kt2 slot

            def normalize(h, qt, pso):
                hj, hp = h // 2, 64 * (h % 2)
                den_r = cp.tile([1, 512], F32R, tag="den", bufs=2)
                nc.vector.tensor_copy(den_r[:], pso[64:65, :])
                psb = psyP.tile([128, 512], F32, tag="psy")
                nc.tensor.matmul(
                    psb[:64], lhsT=ones_r[:], rhs=den_r[:], start=True, stop=True
                )
                rb = cp.tile([64, 512], F32, tag="rb", bufs=2)
                nc.vector.reciprocal_approx_fast(rb[:], psb[:64])
                nc.vector.tensor_tensor(
                    OTs[qt][hp : hp + 64, hj, :],
                    pso[:64],
                    rb[:],
                    mybir.AluOpType.mult,
                )

            def outproj_unit(qt512, sub, e):
                q0 = qt512 * 512 + sub * 128
                psy = psyP.tile([128, 512], F32, tag="psy")
                for cj in range(2):
                    nc.tensor.matmul(
                        psy[:],
                        lhsT=OTs[qt512][:, cj, sub * 128 : (sub + 1) * 128],
                        rhs=WoT[:, cj, e * 512 : (e + 1) * 512],
                        start=(cj == 0),
                        stop=(cj == 1),
                    )
                y_sb = yp.tile([128, 512], F16, tag="y")
                nc.vector.tensor_copy(y_sb[:], psy[:])
                nc.sync.dma_start(
                    out_d[q0 : q0 + 128, e * 512 : (e + 1) * 512], y_sb[:]
                )

            pend_pv = None  # (pso, p_sb, entries, h, qt, is_last)

            def emit_pv(state):
                pso, p_sb, entries, h, qt, is_last = state
                for kt, w, qoff, poff, st, sp in entries:
                    nc.tensor.matmul(
                        pso[:65, qoff : qoff + w],
                        lhsT=Vps[kt // 4][:, kt % 4, h, :],
                        rhs=p_sb[:, poff : poff + w],
                        start=st,
                        stop=sp,
                    )
                if is_last:
                    deferred.append(("norm", (h, qt, pso)))
                    if h == GH - 1:
                        for sub in range(4):
                            for e in range(2):
                                deferred.append(("proj", (qt, sub, e)))

            def pop_deferred():
                if deferred:
                    kind, args = deferred.pop(0)
                    if kind == "norm":
                        normalize(*args)
                    else:
                        outproj_unit(*args)

            for qt in range(QT_TILES):
                # pair list: entries (kt, width, qoff, poff); diagonal first
                if qt == 0:
                    pairs = [
                        [(0, 512, 0, 0), (1, 512, 0, 512)],
                        [(2, 512, 0, 0), (3, 512, 0, 512)],
                    ]
                    full_mask = True
                else:
                    d0 = 4 * qt
                    pairs = [
                        [(d0, 512, 0, 0), (d0 + 1, 384, 128, 512)],
                        [(d0 + 2, 256, 256, 0), (d0 + 3, 128, 384, 256)],
                    ]
                    for m in range(2 * qt):
                        pairs.append(
                            [(2 * m, 512, 0, 0), (2 * m + 1, 512, 0, 512)]
                        )
                    full_mask = False
                n_pairs = len(pairs)
                for h in range(GH):
                    pso = psoP.tile([128, 512], F32, tag="pso")
                    first_pv = True
                    for pi, pair in enumerate(pairs):
                        totw = sum(p[1] for p in pair)
                        pss = pssP.tile([128, 1024], F32, tag="pss")
                        for kt, w, qoff, poff in pair:
                            nc.tensor.matmul(
                                pss[:, poff : poff + w],
                                lhsT=KTzs[kt // 4][
                                    :, h, (kt % 4) * 128 : (kt % 4 + 1) * 128
                                ],
                                rhs=QT[:, h // 2, qt * 512 + qoff : qt * 512 + qoff + w],
                                start=True,
                                stop=True,
                            )
                        p_sb = cp.tile([128, 1024], F16, tag="p", bufs=5)
                        nc.scalar.activation(
                            p_sb[:, 0:totw],
                            pss[:, 0:totw],
                            mybir.ActivationFunctionType.Exp,
                            scale=0.125,
                        )
                        for kt, w, qoff, poff in pair:
                            if kt < 4 * qt:
                                continue  # off-diagonal: no mask
                            if full_mask:
                                nc.gpsimd.affine_select(
                                    out=p_sb[:, poff : poff + w],
                                    in_=p_sb[:, poff : poff + w],
                                    pattern=[[1, w]],
                                    compare_op=mybir.AluOpType.is_ge,
                                    fill=0.0,
                                    base=qt * 512 - kt * 128,
                                    channel_multiplier=-1,
                                )
                            else:
                                # truncated: only first 128 cols can violate
                                nc.gpsimd.affine_select(
                                    out=p_sb[:, poff : poff + 128],
                                    in_=p_sb[:, poff : poff + 128],
                                    pattern=[[1, 128]],
                                    compare_op=mybir.AluOpType.is_ge,
                                    fill=0.0,
                                    base=0,
                                    channel_multiplier=-1,
                                )
                        if pend_pv is not None:
                            emit_pv(pend_pv)
                            pop_deferred()
                        entries = []
                        for kt, w, qoff, poff in pair:
                            entries.append(
                                (kt, w, qoff, poff, first_pv, pi == n_pairs - 1 and kt == pair[-1][0])
                            )
                            first_pv = False
                        pend_pv = (pso, p_sb, entries, h, qt, pi == n_pairs - 1)
            emit_pv(pend_pv)
            while deferred:
                pop_deferred()

    nc.compile()
    return nc


_NC_CACHE = None


def _get_program():
    global _NC_CACHE
    if _NC_CACHE is None:
        _NC_CACHE = _build_program()
    return _NC_CACHE


def _run(in_maps, trace=False, **kw):
    nc = _get_program()
    return run_bass_kernel_spmd(nc, in_maps, list(range(NCORES)), trace=trace, **kw)


def _t16(a):
    return np.ascontiguousarray(np.asarray(a).T.astype(np.float16, order="C"))


def _make_in_maps(x, Wq, bq, Wk, bk, Wv, bv, Wo, bo):
    xts = [_t16(np.asarray(x)[b]) for b in range(B)]  # [D, L] f16
    in_maps = []
    for core in range(NCORES):
        b, g = divmod(core, 4)
        s = slice(g * C, (g + 1) * C)
        in_maps.append(
            {
                "xt": xts[b],
                "wqt": _t16(np.asarray(Wq)[s, :]),  # [D, C]
                "wkt": _t16(np.asarray(Wk)[s, :]),
                "wvt": _t16(np.asarray(Wv)[s, :]),
                "wot": _t16(np.asarray(Wo)[:, s]),  # [C, D]
                "bq": np.ascontiguousarray(np.asarray(bq)[s], dtype=np.float32),
                "bk": np.ascontiguousarray(np.asarray(bk)[s], dtype=np.float32),
                "bv": np.tile(np.asarray(bv)[s].astype(np.float16), 4),
            }
        )
    return in_maps


def kernel(x, Wq, bq, Wk, bk, Wv, bv, Wo, bo, _trace=False, _trace_out=None, _tmpdir=None):
    in_maps = _make_in_maps(x, Wq, bq, Wk, bk, Wv, bv, Wo, bo)
    res = _run(in_maps, trace=_trace, tmpdir=_tmpdir)
    if _trace_out is not None:
        _trace_out.append(res)
    bo = np.asarray(bo, dtype=np.float32)
    out = np.empty((B, L, D), dtype=np.float32)
    for b in range(B):
        acc = res.results[4 * b]["out"].astype(np.float32)
        for g in range(1, 4):
            acc = acc + res.results[4 * b + g]["out"].astype(np.float32)
        out[b] = acc + bo[None, :]
    return out
